# revision 27
# baseline (speedup 1.0000x reference)
"""Trainium2 Bass kernel for nn_BasicLSTM: (B,T,N,C) shared-weight LSTM -> FC.

Strategy (data parallel over 8 cores, B=64 -> 8 batches/core). The kernel is
ScalarE(ACT)-throughput-bound (~0.81ns per free-dim column, all activations),
so the design minimizes ACT columns and ACT instruction count:
  - seqs = 8*1370 = 10960 independent (b,n) sequences per core, T=12, C=8,
    H=64. Layout "gates on partitions, sequences on free dim"; 11 pairs of
    two 512-seq blocks (block0 -> partitions 0:64, block1 -> 64:128).
  - Per (pair, step): 8 matmuls (4 gates x 2 blocks, lhsT = [W_hh.T; W_ih.T;
    b], 73x64) fill ONE 4-bank PSUM tile [i|f|o|2g]; a single wide Sigmoid
    (FD 2048) produces all four gates: tanh(g) = 2*sigmoid(2g) - 1 with the
    g weights host-prescaled by 2 (the affine is a 4x-mode DVE tensor_scalar).
  - Cell state for ALL pairs lives in one big SBUF buffer; tanh(c) runs as 2
    wide chunk instructions per step (vs 11 narrow ones), software-pipelined
    so each chunk's producers finished a couple of sigmoids earlier.
  - h = sigmoid(o)*tanh(c) is written by DVE directly into the next step's
    rhs tile [h(0:64); x_t(64:72); ones(72)]; block1's h moves partitions
    via a SBUF->SBUF DMA (DVE lanes cannot cross partition offsets; matmul
    lhsT/rhs base partitions must be 0 for K>64, so no flipped layout).
  - x arrives host-pretransposed as (9, T, seqs) bf16 (channel 8 = ones row
    carries the gate biases through the contraction); one x-DMA per pair.
  - Last step: per-pair tanh(c) + FC (y = W_fc @ h_T, K=64, bias via a
    per-partition tensor_scalar) pipelined 4+ pairs behind so the h-store
    DMA chain never stalls the tensor queue; block1's FC reads h directly
    from its SBUF tile with a base-64 lhsT copy of W_fc.
  - Ramp: small-weight + first-x DMAs lead the sync queue; 3 warm-up
    matmuls spin the PE toward max pstate under the DMA shadow.
Measured: 374us (baseline) -> ~347us; rel err 6.6e-3 (fp32 ref, bf16 data).
"""

import os
from contextlib import ExitStack

import numpy as np

import concourse.bass as bass
import concourse.mybir as mybir
import concourse.tile as tile
from concourse import bacc
from concourse.bass_utils import run_bass_kernel_spmd
from concourse.tile import add_dep_helper

B, T, N, C, H = 64, 12, 1370, 8, 64
NCORES = 8
BPC = B // NCORES          # batches per core
SEQS = BPC * N             # 10960 sequences per core
S = 512                    # sequence block size (free dim per matmul)
KH = H                     # 64 rows of h in rhs
KX = C + 1                 # 8 x-channels + ones row
K = KH + KX                # 73
G4 = 4 * H                 # 256

BF16 = mybir.dt.bfloat16
F32 = mybir.dt.float32
CDT = BF16  # cell-state dtype (bf16 doubles DVE throughput on the c update)
NPBF16 = mybir.dt.np(BF16)

AF = mybir.ActivationFunctionType


def _blocks(seqs: int, s: int):
    """Blocks of width <= s covering seqs with the remainder split evenly
    over the last two blocks (keeps every ACT/DVE op at its true width
    instead of streaming padding)."""
    nfull = seqs // s
    rem = seqs - nfull * s
    if rem == 0:
        widths = [s] * nfull
    else:
        # steal one full block so the tail splits into two equal-ish blocks
        tail = s + rem
        widths = [s] * (nfull - 1) + [(tail + 1) // 2, tail // 2]
    out = []
    c0 = 0
    for w in widths:
        out.append((c0, w))
        c0 += w
    return out


def _fence(nc, producers):
    """The TRN2 Matmult ISA slot holds a single sync-wait, but the first
    matmul of a step naturally depends on 3-4 engines (ACT psum release, DVE
    h-write, DMA x / h-move). Funnel those deps through a chain of SyncE nops
    (one wait each); callers then depend only on the returned nop."""
    last = None
    seen = set()
    for p in producers:
        if p is None:
            continue
        pi = p.ins if hasattr(p, "ins") else p
        if id(pi) in seen:
            continue
        seen.add(id(pi))
        n = nc.sync.nop(nofuse=True, hint="depfence")
        add_dep_helper(n.ins, pi, reason="dep fence")
        last = n
    return last


def build_nc(seqs: int = SEQS, s: int = S, use_fence: bool = False) -> bass.Bass:
    nc = bacc.Bacc("TRN2", target_bir_lowering=False, debug=False)

    xin = nc.declare_dram_parameter("xin", [KX, T, seqs], BF16, isOutput=False)
    wg = nc.declare_dram_parameter("wg", [K, G4], BF16, isOutput=False)
    wfc = nc.declare_dram_parameter("wfc", [H + 1, C], BF16, isOutput=False)
    yb = nc.declare_dram_parameter("yb", [128, 1], F32, isOutput=False)
    y = nc.declare_dram_parameter("y", [C, seqs], F32, isOutput=True)

    blocks = _blocks(seqs, s)
    pairs = [blocks[i : i + 2] for i in range(0, len(blocks), 2)]

    with tile.TileContext(nc) as tc, ExitStack() as ctx:
        const = ctx.enter_context(tc.tile_pool(name="const", bufs=1))
        rhsp = ctx.enter_context(tc.tile_pool(name="rhs", bufs=36))
        sgp = ctx.enter_context(tc.tile_pool(name="sg", bufs=12))
        vgp = ctx.enter_context(tc.tile_pool(name="vg", bufs=6))
        igp = ctx.enter_context(tc.tile_pool(name="ig", bufs=6))
        fcpool = ctx.enter_context(tc.tile_pool(name="fcp", bufs=6))
        thcp = ctx.enter_context(tc.tile_pool(name="thcl", bufs=4))
        htp = ctx.enter_context(tc.tile_pool(name="htmp", bufs=7))
        ysp = ctx.enter_context(tc.tile_pool(name="ys", bufs=2))
        # one 4-bank PSUM tile [i|f|o|2g] per pair, double-buffered = all 8
        # banks; the FC tail's pf tiles borrow slots from the same pool
        pifo = ctx.enter_context(tc.tile_pool(name="pifo", bufs=2, space="PSUM"))

        w_sb = const.tile([K, G4], BF16)
        # x-projection rows first: the t=0 matmuls need only rows KH:K, so
        # they can launch before the bulk of the weights arrive; the first
        # pair's t=0 x ride right behind it on the sync queue
        nc.sync.dma_start(out=w_sb[KH:K, :], in_=wg[KH:K, :])

        wfc_sb = const.tile([H + 1, C], BF16)
        # W_fc.T replicated at partitions 64:128: lets the T-1 FC matmul for
        # block1 read h straight out of the ht tile (partitions 64:128),
        # skipping the h-store DMA hop on the critical tail
        wfc64_sb = const.tile([128, C], BF16)
        yb_sb = const.tile([128, 1], F32)
        # cell state + tanh(c) for ALL pairs live in two big SBUF buffers so
        # the per-step tanh(c) runs as 2 wide ACT instructions instead of 11
        # narrow ones (saves ~180ns/inst of ACT access overhead)
        cbuf = const.tile([128, seqs // 2], CDT)
        thcbuf = const.tile([128, seqs // 2], BF16)
        # compact: only block0's h goes through hstore (block1 h is read by
        # the FC straight from its ht tile), indexed by the pair offset
        hstore = const.tile([H, seqs // 2], BF16)
        scratch = const.tile([128, s], BF16)

        # gate column offsets in w_sb: pytorch order i, f, g, o
        WI, WF, WG, WO = 0, H, 2 * H, 3 * H

        MULT = mybir.AluOpType.mult
        ADDOP = mybir.AluOpType.add

        def phase1(st, t):
            """Gate matmuls, one sigmoid over [i|f|o|2g], cell update -> cbuf.

            tanh(g) = 2*sigmoid(2g) - 1; the host pre-scales the g-gate
            weights by 2, so a single wide Sigmoid covers all four gates and
            a cheap 4x-mode tensor_scalar recovers tanh(g) on the DVE."""
            pair, rhs_t, off = st["pair"], st["rhs"], st["off"]
            wd = pair[0][1]  # both blocks in a pair have equal width
            ifo = pifo.tile([128, 4 * s], F32, name="ifo", tag="ifo")

            # matmuls; gate sections bank-aligned at stride s (a PSUM matmul
            # dst must not cross a 2KB bank boundary -- verified wrong on HW
            # otherwise); the g section sits last so the sigmoid can stop at
            # 3*s+wd and skip the tail pair's trailing padding
            first = t == 0 and st is states[0]
            order = ((WI, 0), (WF, 1), (WG, 3), (WO, 2)) if first else (
                (WG, 3), (WI, 0), (WF, 1), (WO, 2))
            half = None
            for gi, (gof, dsec) in enumerate(order):
                for blk, (c0, bwd) in enumerate(pair):
                    pb = 64 * blk
                    b0 = blk * pair[0][1]
                    if t == 0:
                        lh = w_sb[KH:K, gof : gof + H]
                        rh = rhs_t[t][KH:K, b0 : b0 + bwd]
                    else:
                        lh = w_sb[:, gof : gof + H]
                        rh = rhs_t[t][:, b0 : b0 + bwd]
                    nc.tensor.matmul(
                        ifo[pb : pb + 64, dsec * s : dsec * s + bwd], lh, rh
                    )
                if first and gi == 1:
                    # kernel-start only: sigmoid the [i|f] half as soon as its
                    # 4 matmuls land, so ACT starts ~1us earlier
                    half = sgp.tile([128, 4 * s], BF16, name="sg", tag="sg")
                    nc.scalar.activation(
                        half[:, 0 : 2 * s], ifo[:, 0 : 2 * s], AF.Sigmoid
                    )

            if half is not None:
                sg = half
                nc.scalar.activation(
                    sg[:, 2 * s : 3 * s + wd], ifo[:, 2 * s : 3 * s + wd],
                    AF.Sigmoid,
                )
            else:
                sg = sgp.tile([128, 4 * s], BF16, name="sg", tag="sg")
                nc.scalar.activation(
                    sg[:, 0 : 3 * s + wd], ifo[:, 0 : 3 * s + wd], AF.Sigmoid
                )
            st["sg"] = sg
            # tanh(g) = 2*sigmoid(2g) - 1 on the DVE (4x mode)
            vg = vgp.tile([128, s], BF16, name="vg", tag="vg")
            nc.vector.tensor_scalar(
                vg[:, 0:wd], sg[:, 3 * s : 3 * s + wd], 2.0, -1.0, MULT, ADDOP
            )

            c_t = cbuf[:, off : off + wd]
            if t == 0:
                nc.vector.tensor_mul(c_t, sg[:, 0:wd], vg[:, 0:wd])
            else:
                ig = igp.tile([128, s], BF16, name="ig", tag="ig")
                nc.vector.tensor_mul(ig[:, 0:wd], sg[:, 0:wd], vg[:, 0:wd])
                fc = fcpool.tile([128, s], CDT, name="fc", tag="fc")
                nc.vector.tensor_mul(fc[:, 0:wd], sg[:, s : s + wd], c_t)
                nc.vector.tensor_add(c_t, ig[:, 0:wd], fc[:, 0:wd])

        def phase2(st, t, thc=None, thc_off=None):
            """h = sigmoid(o) * tanh(c) into next-step rhs (or hstore)."""
            pair, rhs_t, off, sg = st["pair"], st["rhs"], st["off"], st["sg"]
            if thc is None:
                thc, thc_off = thcbuf, off
            for blk, (c0, wd) in enumerate(pair):
                pb = 64 * blk
                so = sg[pb : pb + 64, 2 * s : 2 * s + wd]
                tc_half = thc[pb : pb + 64, thc_off : thc_off + wd]
                b0 = blk * pair[0][1]
                if blk == 0:
                    dst = (hstore[0:H, off : off + wd] if t == T - 1
                           else rhs_t[t + 1][0:KH, b0 : b0 + wd])
                    nc.vector.tensor_mul(dst, so, tc_half)
                else:
                    ht = htp.tile([128, s], BF16, name="ht", tag="ht")
                    nc.vector.tensor_mul(ht[pb : pb + 64, 0:wd], so, tc_half)
                    if t == T - 1:
                        st["ht"] = ht  # FC reads h directly from here
                    else:
                        nc.sync.dma_start(
                            out=rhs_t[t + 1][0:KH, b0 : b0 + wd],
                            in_=ht[pb : pb + 64, 0:wd],
                        )

        def emit_fc(sts):
            """FC for up to two pairs: one pf tile + one wide bias-add, so
            the PSUM-pool rotation only pays one extra slot per duo. Block0
            reads hstore; block1 reads its ht tile directly (base-64 lhsT)."""
            pf = pifo.tile([128, 4 * s], F32, tag="ifo", name="pf")
            blks = []
            for d, st in enumerate(sts):
                for blk, (c0, wd) in enumerate(st["pair"]):
                    blks.append((c0, wd, blk, d, st))
            for c0, wd, blk, d, st in blks:
                if blk == 0:
                    nc.tensor.matmul(
                        pf[0:C, d * s : d * s + wd],
                        wfc_sb[0:H, :],
                        hstore[:, st["off"] : st["off"] + wd],
                    )
                else:
                    nc.tensor.matmul(
                        pf[32 : 32 + C, d * s : d * s + wd],
                        wfc64_sb[64:128, :],
                        st["ht"][64:128, 0:wd],
                    )
            yt = ysp.tile([128, 4 * s], F32, name="yt", tag="yt")
            nw = len(sts) * s
            nc.vector.tensor_scalar(
                yt[0:40, 0:nw], pf[0:40, 0:nw], yb_sb[0:40, 0:1], None, ADDOP
            )
            for j, (c0, wd, blk, d, st) in enumerate(blks):
                eng = nc.sync if j % 2 == 0 else nc.gpsimd
                eng.dma_start(
                    out=y[:, c0 : c0 + wd],
                    in_=yt[32 * blk : 32 * blk + C, d * s : d * s + wd],
                )

        # process pairs in interleaved groups so several independent
        # recurrences keep every engine busy (and TensorE HAM-warm)
        GROUP = 11
        PREFETCH = 2

        def alloc_rhs(st, t, eng=None):
            pair, rhs_t = st["pair"], st["rhs"]
            pw = sum(w for _, w in pair)
            rt = rhsp.tile([K, 2 * s], BF16, name="rhs", tag="rhs")
            rhs_t[t] = rt
            # one DMA covers both blocks' x (halves the DMA-issue load)
            (eng or nc.gpsimd).dma_start(
                out=rt[KH:K, 0:pw],
                in_=xin[:, t, pair[0][0] : pair[0][0] + pw],
            )

        for g0 in range(0, len(pairs), GROUP):
            grp = pairs[g0 : g0 + GROUP]
            states = []
            off = 0
            for pair in grp:
                states.append({"pair": pair, "rhs": [None] * T, "off": off})
                off += pair[0][1]
            # first pair's t=0 x as sync DMA #2 (right behind the small wg)
            alloc_rhs(states[0], 0, nc.sync)
            # remaining const DMAs now that the critical two are in flight
            nc.sync.dma_start(out=w_sb[0:KH, :], in_=wg[0:KH, :])
            nc.sync.dma_start(out=wfc_sb[:, :], in_=wfc[:, :])
            nc.sync.dma_start(out=wfc64_sb[64:128, :], in_=wfc[0:H, :])
            nc.sync.dma_start(out=yb_sb[:, :], in_=yb[:, :])
            # PE warm-up (gpsimd memset frees the DVE and starts ~1us sooner)
            nc.gpsimd.memset(scratch[:, :], 1.0)
            nc.scalar.activation(scratch[0:1, 0:8], scratch[0:1, 0:8], AF.Sigmoid)
            wmup = pifo.tile([128, 4 * s], F32, name="wmup", tag="ifo")
            for wi in range(3):
                nc.tensor.matmul(
                    wmup[0:64, 0:s], scratch[0:73, 0:64], scratch[0:73, 0:s]
                )
            for pi, st in enumerate(states):
                for t in range(PREFETCH):
                    if (pi, t) != (0, 0):
                        alloc_rhs(st, t)
            # Software-pipelined schedule. The batched tanh(c) is split in two
            # chunks, and each chunk's ACT instruction is emitted one-or-two
            # sigmoids AFTER its last producer pair so the DVE cell-update
            # latency hides behind ACT work (no ACT gap waiting on the adds).
            # The first two pairs of step t+1 are emitted before chunk B of
            # step t for the same reason.
            mid = (len(states) + 1) // 2
            A, Bc = states[:mid], states[mid:]

            def emit_phase1(st, t):
                if t + PREFETCH < T:
                    alloc_rhs(st, t + PREFETCH)
                phase1(st, t)

            def chunk_tanh(grp, t):
                lo = grp[0]["off"]
                hi = grp[-1]["off"] + grp[-1]["pair"][0][1]
                nc.scalar.activation(thcbuf[:, lo:hi], cbuf[:, lo:hi], AF.Tanh)

            for t in range(T - 1):
                for st in states[0 if t == 0 else 2 : mid]:
                    emit_phase1(st, t)
                emit_phase1(states[mid], t)  # p_mid before chunk A's tanh
                chunk_tanh(A, t)
                for st in A:
                    phase2(st, t)
                for st in states[mid + 1 :]:
                    emit_phase1(st, t)
                for st in states[0:2]:  # head of step t+1 before chunk B
                    emit_phase1(st, t + 1)
                chunk_tanh(Bc, t)
                for st in Bc:
                    phase2(st, t)
            # last step: per-pair tanh(c), pipelined with lags -- tanh(p) rides
            # one sigmoid behind phase1(p+1) (so the DVE cell update is done),
            # and FC(p) rides FC_LAG pairs behind (so its h-store DMA chain,
            # incl the ~900ns sem propagation, is complete -> no tensor stall)
            FC_LAG = 4

            def tanh_phase2(st):
                wd = st["pair"][0][1]
                thc = thcp.tile([128, s], BF16, name="thc", tag="thc")
                nc.scalar.activation(
                    thc[:, 0:wd],
                    cbuf[:, st["off"] : st["off"] + wd],
                    AF.Tanh,
                )
                phase2(st, T - 1, thc=thc, thc_off=0)

            # p0/p1 phase1 ran during step T-2's tail; chunk A keeps the
            # batched tanh (its pairs' FCs ride behind anyway), the rest get
            # per-pair tanh so each h_T -> FC drains while later pairs stream
            for i in range(2, mid + 1):
                phase1(states[i], T - 1)
            chunk_tanh(A, T - 1)
            for st in A:
                phase2(st, T - 1)
            for i in range(mid + 1, len(states)):
                phase1(states[i], T - 1)
                tanh_phase2(states[i - 1])
                if i == 7:
                    emit_fc(states[0:4])
                elif i == 9:
                    emit_fc(states[4:8])
                elif i == 10:
                    emit_fc(states[8:10])
            tanh_phase2(states[-1])
            emit_fc(states[10:11])


    nc.compile()
    return nc


def prep_inputs(x, W_ih, W_hh, b_ih, b_hh, W_fc, b_fc, seqs=SEQS, ncores=NCORES):
    """Host-side shard + transpose + weight packing. Returns in_maps."""
    x = np.asarray(x, dtype=np.float32)
    W_ih = np.asarray(W_ih, dtype=np.float32)
    W_hh = np.asarray(W_hh, dtype=np.float32)
    b = np.asarray(b_ih, dtype=np.float32) + np.asarray(b_hh, dtype=np.float32)
    W_fc = np.asarray(W_fc, dtype=np.float32)
    b_fc = np.asarray(b_fc, dtype=np.float32)

    wg = np.zeros((K, G4), dtype=np.float32)
    for g in range(4):
        rows = slice(H * g, H * g + H)
        wg[0:KH, H * g : H * g + H] = W_hh[rows, :].T
        wg[KH : KH + C, H * g : H * g + H] = W_ih[rows, :].T
        wg[K - 1, H * g : H * g + H] = b[rows]
    # g-gate (pytorch gate index 2) pre-scaled by 2: the kernel evaluates
    # tanh(g) as 2*sigmoid(2g)-1 inside one wide Sigmoid instruction
    wg[:, 2 * H : 3 * H] *= 2.0
    wg = wg.astype(NPBF16)

    wfc = np.concatenate([W_fc.T, b_fc[None, :]], axis=0).astype(NPBF16)  # (65, 8)
    yb = np.zeros((128, 1), dtype=np.float32)
    yb[0:C, 0] = b_fc
    yb[32 : 32 + C, 0] = b_fc

    bpc = x.shape[0] // ncores
    in_maps = []
    for k in range(ncores):
        xc = x[k * bpc : (k + 1) * bpc]              # (bpc, T, N, C)
        xt = xc.transpose(3, 1, 0, 2).reshape(C, T, seqs)
        xext = np.empty((KX, T, seqs), dtype=NPBF16)
        xext[0:C] = xt.astype(NPBF16)
        xext[C] = np.ones((T, seqs), dtype=NPBF16)
        in_maps.append({"xin": xext, "wg": wg, "wfc": wfc, "yb": yb})
    return in_maps


_CACHE = {}


def _get_nc():
    if "nc" not in _CACHE:
        _CACHE["nc"] = build_nc()
    return _CACHE["nc"]


def kernel(x, W_ih, W_hh, b_ih, b_hh, W_fc, b_fc, **run_kwargs):
    nc = _get_nc()
    in_maps = prep_inputs(x, W_ih, W_hh, b_ih, b_hh, W_fc, b_fc)
    res = run_bass_kernel_spmd(nc, in_maps, list(range(NCORES)), **run_kwargs)
    outs = res.results
    ys = []
    for k in range(NCORES):
        yk = np.asarray(outs[k]["y"])               # (C, SEQS) f32
        ys.append(yk.T.reshape(BPC, N, C))
    y = np.concatenate(ys, axis=0)                  # (B, N, C)
    if run_kwargs.get("trace"):
        _CACHE["last_result"] = res
    return y.astype(np.float32)



# revision 28
# speedup vs baseline: 1.0089x; 1.0089x over previous
"""Trainium2 Bass kernel for nn_BasicLSTM: (B,T,N,C) shared-weight LSTM -> FC.

Strategy (data parallel over 8 cores, B=64 -> 8 batches/core). The kernel is
ScalarE(ACT)-throughput-bound (~0.81ns per free-dim column, all activations),
so the design minimizes ACT columns and ACT instruction count:
  - seqs = 8*1370 = 10960 independent (b,n) sequences per core, T=12, C=8,
    H=64. Layout "gates on partitions, sequences on free dim"; 11 pairs of
    two 512-seq blocks (block0 -> partitions 0:64, block1 -> 64:128).
  - Per (pair, step): 8 matmuls (4 gates x 2 blocks, lhsT = [W_hh.T; W_ih.T;
    b], 73x64) fill ONE 4-bank PSUM tile [i|f|o|2g]; a single wide Sigmoid
    (FD 2048) produces all four gates: tanh(g) = 2*sigmoid(2g) - 1 with the
    g weights host-prescaled by 2 (the affine is a 4x-mode DVE tensor_scalar).
  - Cell state for ALL pairs lives in one big SBUF buffer; tanh(c) runs as 2
    wide chunk instructions per step (vs 11 narrow ones), software-pipelined
    so each chunk's producers finished a couple of sigmoids earlier.
  - h = sigmoid(o)*tanh(c) is written by DVE directly into the next step's
    rhs tile [h(0:64); x_t(64:72); ones(72)]; block1's h moves partitions
    via a SBUF->SBUF DMA (DVE lanes cannot cross partition offsets; matmul
    lhsT/rhs base partitions must be 0 for K>64, so no flipped layout).
  - x arrives host-pretransposed as (9, T, seqs) bf16 (channel 8 = ones row
    carries the gate biases through the contraction); one x-DMA per pair.
  - Last step: per-pair tanh(c) + FC (y = W_fc @ h_T, K=64, bias via a
    per-partition tensor_scalar) pipelined 4+ pairs behind so the h-store
    DMA chain never stalls the tensor queue; block1's FC reads h directly
    from its SBUF tile with a base-64 lhsT copy of W_fc.
  - Ramp: small-weight + first-x DMAs lead the sync queue; 3 warm-up
    matmuls spin the PE toward max pstate under the DMA shadow.
Measured: 374us (baseline) -> ~347us; rel err 6.6e-3 (fp32 ref, bf16 data).
"""

import os
from contextlib import ExitStack

import numpy as np

import concourse.bass as bass
import concourse.mybir as mybir
import concourse.tile as tile
from concourse import bacc
from concourse.bass_utils import run_bass_kernel_spmd
from concourse.tile import add_dep_helper

B, T, N, C, H = 64, 12, 1370, 8, 64
NCORES = 8
BPC = B // NCORES          # batches per core
SEQS = BPC * N             # 10960 sequences per core
S = 512                    # sequence block size (free dim per matmul)
KH = H                     # 64 rows of h in rhs
KX = C + 1                 # 8 x-channels + ones row
K = KH + KX                # 73
G4 = 4 * H                 # 256

BF16 = mybir.dt.bfloat16
F32 = mybir.dt.float32
CDT = BF16  # cell-state dtype (bf16 doubles DVE throughput on the c update)
NPBF16 = mybir.dt.np(BF16)

AF = mybir.ActivationFunctionType


def _blocks(seqs: int, s: int):
    """Blocks of width <= s covering seqs with the remainder split evenly
    over the last two blocks (keeps every ACT/DVE op at its true width
    instead of streaming padding)."""
    nfull = seqs // s
    rem = seqs - nfull * s
    if rem == 0:
        widths = [s] * nfull
    else:
        # steal one full block so the tail splits into two equal-ish blocks
        tail = s + rem
        widths = [s] * (nfull - 1) + [(tail + 1) // 2, tail // 2]
    out = []
    c0 = 0
    for w in widths:
        out.append((c0, w))
        c0 += w
    return out


def _fence(nc, producers):
    """The TRN2 Matmult ISA slot holds a single sync-wait, but the first
    matmul of a step naturally depends on 3-4 engines (ACT psum release, DVE
    h-write, DMA x / h-move). Funnel those deps through a chain of SyncE nops
    (one wait each); callers then depend only on the returned nop."""
    last = None
    seen = set()
    for p in producers:
        if p is None:
            continue
        pi = p.ins if hasattr(p, "ins") else p
        if id(pi) in seen:
            continue
        seen.add(id(pi))
        n = nc.sync.nop(nofuse=True, hint="depfence")
        add_dep_helper(n.ins, pi, reason="dep fence")
        last = n
    return last


def build_nc(seqs: int = SEQS, s: int = S, use_fence: bool = False) -> bass.Bass:
    nc = bacc.Bacc("TRN2", target_bir_lowering=False, debug=False)

    xin = nc.declare_dram_parameter("xin", [KX, T, seqs], BF16, isOutput=False)
    wg = nc.declare_dram_parameter("wg", [K, G4], BF16, isOutput=False)
    wfc = nc.declare_dram_parameter("wfc", [H + 1, C], BF16, isOutput=False)
    yb = nc.declare_dram_parameter("yb", [128, 1], F32, isOutput=False)
    y = nc.declare_dram_parameter("y", [C, seqs], F32, isOutput=True)

    blocks = _blocks(seqs, s)
    pairs = [blocks[i : i + 2] for i in range(0, len(blocks), 2)]

    with tile.TileContext(nc) as tc, ExitStack() as ctx:
        const = ctx.enter_context(tc.tile_pool(name="const", bufs=1))
        rhsp = ctx.enter_context(tc.tile_pool(name="rhs", bufs=36))
        sgp = ctx.enter_context(tc.tile_pool(name="sg", bufs=12))
        vgp = ctx.enter_context(tc.tile_pool(name="vg", bufs=6))
        igp = ctx.enter_context(tc.tile_pool(name="ig", bufs=6))
        fcpool = ctx.enter_context(tc.tile_pool(name="fcp", bufs=6))
        thcp = ctx.enter_context(tc.tile_pool(name="thcl", bufs=4))
        htp = ctx.enter_context(tc.tile_pool(name="htmp", bufs=7))
        ysp = ctx.enter_context(tc.tile_pool(name="ys", bufs=2))
        # one 4-bank PSUM tile [i|f|o|2g] per pair, double-buffered = all 8
        # banks; the FC tail's pf tiles borrow slots from the same pool
        pifo = ctx.enter_context(tc.tile_pool(name="pifo", bufs=2, space="PSUM"))

        w_sb = const.tile([K, G4], BF16)
        # x-projection rows first: the t=0 matmuls need only rows KH:K, so
        # they can launch before the bulk of the weights arrive; the first
        # pair's t=0 x ride right behind it on the sync queue
        nc.sync.dma_start(out=w_sb[KH:K, :], in_=wg[KH:K, :])

        wfc_sb = const.tile([H + 1, C], BF16)
        # W_fc.T replicated at partitions 64:128: lets the T-1 FC matmul for
        # block1 read h straight out of the ht tile (partitions 64:128),
        # skipping the h-store DMA hop on the critical tail
        wfc64_sb = const.tile([128, C], BF16)
        yb_sb = const.tile([128, 1], F32)
        # cell state + tanh(c) for ALL pairs live in two big SBUF buffers so
        # the per-step tanh(c) runs as 2 wide ACT instructions instead of 11
        # narrow ones (saves ~180ns/inst of ACT access overhead)
        cbuf = const.tile([128, seqs // 2], CDT)
        thcbuf = const.tile([128, seqs // 2], BF16)
        # compact: only block0's h goes through hstore (block1 h is read by
        # the FC straight from its ht tile), indexed by the pair offset
        hstore = const.tile([H, seqs // 2], BF16)
        scratch = const.tile([128, s], BF16)

        # gate column offsets in w_sb: pytorch order i, f, g, o
        WI, WF, WG, WO = 0, H, 2 * H, 3 * H

        MULT = mybir.AluOpType.mult
        ADDOP = mybir.AluOpType.add

        def phase1(st, t):
            """Gate matmuls, one sigmoid over [i|f|o|2g], cell update -> cbuf.

            tanh(g) = 2*sigmoid(2g) - 1; the host pre-scales the g-gate
            weights by 2, so a single wide Sigmoid covers all four gates and
            a cheap 4x-mode tensor_scalar recovers tanh(g) on the DVE."""
            pair, rhs_t, off = st["pair"], st["rhs"], st["off"]
            wd = pair[0][1]  # both blocks in a pair have equal width
            ifo = pifo.tile([128, 4 * s], F32, name="ifo", tag="ifo")

            # matmuls; gate sections bank-aligned at stride s (a PSUM matmul
            # dst must not cross a 2KB bank boundary -- verified wrong on HW
            # otherwise); the g section sits last so the sigmoid can stop at
            # 3*s+wd and skip the tail pair's trailing padding
            for gof, dsec in ((WG, 3), (WI, 0), (WF, 1), (WO, 2)):
                for blk, (c0, bwd) in enumerate(pair):
                    pb = 64 * blk
                    b0 = blk * pair[0][1]
                    if t == 0:
                        lh = w_sb[KH:K, gof : gof + H]
                        rh = rhs_t[t][KH:K, b0 : b0 + bwd]
                    else:
                        lh = w_sb[:, gof : gof + H]
                        rh = rhs_t[t][:, b0 : b0 + bwd]
                    nc.tensor.matmul(
                        ifo[pb : pb + 64, dsec * s : dsec * s + bwd], lh, rh
                    )

            sg = sgp.tile([128, 4 * s], BF16, name="sg", tag="sg")
            nc.scalar.activation(
                sg[:, 0 : 3 * s + wd], ifo[:, 0 : 3 * s + wd], AF.Sigmoid
            )
            st["sg"] = sg
            # tanh(g) = 2*sigmoid(2g) - 1 on the DVE (4x mode)
            vg = vgp.tile([128, s], BF16, name="vg", tag="vg")
            nc.vector.tensor_scalar(
                vg[:, 0:wd], sg[:, 3 * s : 3 * s + wd], 2.0, -1.0, MULT, ADDOP
            )

            c_t = cbuf[:, off : off + wd]
            if t == 0:
                nc.vector.tensor_mul(c_t, sg[:, 0:wd], vg[:, 0:wd])
            else:
                ig = igp.tile([128, s], BF16, name="ig", tag="ig")
                nc.vector.tensor_mul(ig[:, 0:wd], sg[:, 0:wd], vg[:, 0:wd])
                fc = fcpool.tile([128, s], CDT, name="fc", tag="fc")
                nc.vector.tensor_mul(fc[:, 0:wd], sg[:, s : s + wd], c_t)
                nc.vector.tensor_add(c_t, ig[:, 0:wd], fc[:, 0:wd])

        def phase2(st, t, thc=None, thc_off=None):
            """h = sigmoid(o) * tanh(c) into next-step rhs (or hstore)."""
            pair, rhs_t, off, sg = st["pair"], st["rhs"], st["off"], st["sg"]
            if thc is None:
                thc, thc_off = thcbuf, off
            for blk, (c0, wd) in enumerate(pair):
                pb = 64 * blk
                so = sg[pb : pb + 64, 2 * s : 2 * s + wd]
                tc_half = thc[pb : pb + 64, thc_off : thc_off + wd]
                b0 = blk * pair[0][1]
                if blk == 0:
                    dst = (hstore[0:H, off : off + wd] if t == T - 1
                           else rhs_t[t + 1][0:KH, b0 : b0 + wd])
                    nc.vector.tensor_mul(dst, so, tc_half)
                else:
                    ht = htp.tile([128, s], BF16, name="ht", tag="ht")
                    nc.vector.tensor_mul(ht[pb : pb + 64, 0:wd], so, tc_half)
                    if t == T - 1:
                        st["ht"] = ht  # FC reads h directly from here
                    else:
                        nc.sync.dma_start(
                            out=rhs_t[t + 1][0:KH, b0 : b0 + wd],
                            in_=ht[pb : pb + 64, 0:wd],
                        )

        def emit_fc(sts):
            """FC for up to two pairs: one pf tile + one wide bias-add, so
            the PSUM-pool rotation only pays one extra slot per duo. Block0
            reads hstore; block1 reads its ht tile directly (base-64 lhsT)."""
            pf = pifo.tile([128, 4 * s], F32, tag="ifo", name="pf")
            blks = []
            for d, st in enumerate(sts):
                for blk, (c0, wd) in enumerate(st["pair"]):
                    blks.append((c0, wd, blk, d, st))
            for c0, wd, blk, d, st in blks:
                if blk == 0:
                    nc.tensor.matmul(
                        pf[0:C, d * s : d * s + wd],
                        wfc_sb[0:H, :],
                        hstore[:, st["off"] : st["off"] + wd],
                    )
                else:
                    nc.tensor.matmul(
                        pf[32 : 32 + C, d * s : d * s + wd],
                        wfc64_sb[64:128, :],
                        st["ht"][64:128, 0:wd],
                    )
            yt = ysp.tile([128, 4 * s], F32, name="yt", tag="yt")
            nw = len(sts) * s
            nc.vector.tensor_scalar(
                yt[0:40, 0:nw], pf[0:40, 0:nw], yb_sb[0:40, 0:1], None, ADDOP
            )
            for j, (c0, wd, blk, d, st) in enumerate(blks):
                eng = nc.sync if j % 2 == 0 else nc.gpsimd
                eng.dma_start(
                    out=y[:, c0 : c0 + wd],
                    in_=yt[32 * blk : 32 * blk + C, d * s : d * s + wd],
                )

        # process pairs in interleaved groups so several independent
        # recurrences keep every engine busy (and TensorE HAM-warm)
        GROUP = 11
        PREFETCH = 2

        def alloc_rhs(st, t, eng=None):
            pair, rhs_t = st["pair"], st["rhs"]
            pw = sum(w for _, w in pair)
            rt = rhsp.tile([K, 2 * s], BF16, name="rhs", tag="rhs")
            rhs_t[t] = rt
            # one DMA covers both blocks' x (halves the DMA-issue load)
            (eng or nc.gpsimd).dma_start(
                out=rt[KH:K, 0:pw],
                in_=xin[:, t, pair[0][0] : pair[0][0] + pw],
            )

        for g0 in range(0, len(pairs), GROUP):
            grp = pairs[g0 : g0 + GROUP]
            states = []
            off = 0
            for pair in grp:
                states.append({"pair": pair, "rhs": [None] * T, "off": off})
                off += pair[0][1]
            # first pair's t=0 x as sync DMA #2 (right behind the small wg)
            alloc_rhs(states[0], 0, nc.sync)
            # remaining const DMAs now that the critical two are in flight
            nc.sync.dma_start(out=w_sb[0:KH, :], in_=wg[0:KH, :])
            nc.sync.dma_start(out=wfc_sb[:, :], in_=wfc[:, :])
            nc.sync.dma_start(out=wfc64_sb[64:128, :], in_=wfc[0:H, :])
            nc.sync.dma_start(out=yb_sb[:, :], in_=yb[:, :])
            # PE warm-up (gpsimd memset frees the DVE and starts ~1us sooner)
            nc.gpsimd.memset(scratch[:, :], 1.0)
            nc.scalar.activation(scratch[0:1, 0:8], scratch[0:1, 0:8], AF.Sigmoid)
            wmup = pifo.tile([128, 4 * s], F32, name="wmup", tag="ifo")
            for wi in range(3):
                nc.tensor.matmul(
                    wmup[0:64, 0:s], scratch[0:73, 0:64], scratch[0:73, 0:s]
                )
            for pi, st in enumerate(states):
                for t in range(PREFETCH):
                    if (pi, t) != (0, 0):
                        alloc_rhs(st, t)
            # Software-pipelined schedule. The batched tanh(c) is split in two
            # chunks, and each chunk's ACT instruction is emitted one-or-two
            # sigmoids AFTER its last producer pair so the DVE cell-update
            # latency hides behind ACT work (no ACT gap waiting on the adds).
            # The first two pairs of step t+1 are emitted before chunk B of
            # step t for the same reason.
            mid = (len(states) + 1) // 2
            A, Bc = states[:mid], states[mid:]

            def emit_phase1(st, t):
                if t + PREFETCH < T:
                    alloc_rhs(st, t + PREFETCH)
                phase1(st, t)

            def chunk_tanh(grp, t):
                lo = grp[0]["off"]
                hi = grp[-1]["off"] + grp[-1]["pair"][0][1]
                nc.scalar.activation(thcbuf[:, lo:hi], cbuf[:, lo:hi], AF.Tanh)

            for t in range(T - 1):
                for st in states[0 if t == 0 else 2 : mid]:
                    emit_phase1(st, t)
                emit_phase1(states[mid], t)  # p_mid before chunk A's tanh
                chunk_tanh(A, t)
                for st in A:
                    phase2(st, t)
                for st in states[mid + 1 :]:
                    emit_phase1(st, t)
                for st in states[0:2]:  # head of step t+1 before chunk B
                    emit_phase1(st, t + 1)
                chunk_tanh(Bc, t)
                for st in Bc:
                    phase2(st, t)
            # last step: per-pair tanh(c), pipelined with lags -- tanh(p) rides
            # one sigmoid behind phase1(p+1) (so the DVE cell update is done),
            # and FC(p) rides FC_LAG pairs behind (so its h-store DMA chain,
            # incl the ~900ns sem propagation, is complete -> no tensor stall)
            FC_LAG = 4

            def tanh_phase2(st):
                wd = st["pair"][0][1]
                thc = thcp.tile([128, s], BF16, name="thc", tag="thc")
                nc.scalar.activation(
                    thc[:, 0:wd],
                    cbuf[:, st["off"] : st["off"] + wd],
                    AF.Tanh,
                )
                phase2(st, T - 1, thc=thc, thc_off=0)

            tanh_phase2(states[0])  # p0/p1 phase1 ran during step T-2's tail
            for i in range(2, len(states)):
                phase1(states[i], T - 1)
                tanh_phase2(states[i - 1])
                if i == 7:
                    emit_fc(states[0:4])
                elif i == 9:
                    emit_fc(states[4:8])
                elif i == 10:
                    emit_fc(states[8:10])
            tanh_phase2(states[-1])
            emit_fc(states[10:11])


    nc.compile()
    return nc


def prep_inputs(x, W_ih, W_hh, b_ih, b_hh, W_fc, b_fc, seqs=SEQS, ncores=NCORES):
    """Host-side shard + transpose + weight packing. Returns in_maps."""
    x = np.asarray(x, dtype=np.float32)
    W_ih = np.asarray(W_ih, dtype=np.float32)
    W_hh = np.asarray(W_hh, dtype=np.float32)
    b = np.asarray(b_ih, dtype=np.float32) + np.asarray(b_hh, dtype=np.float32)
    W_fc = np.asarray(W_fc, dtype=np.float32)
    b_fc = np.asarray(b_fc, dtype=np.float32)

    wg = np.zeros((K, G4), dtype=np.float32)
    for g in range(4):
        rows = slice(H * g, H * g + H)
        wg[0:KH, H * g : H * g + H] = W_hh[rows, :].T
        wg[KH : KH + C, H * g : H * g + H] = W_ih[rows, :].T
        wg[K - 1, H * g : H * g + H] = b[rows]
    # g-gate (pytorch gate index 2) pre-scaled by 2: the kernel evaluates
    # tanh(g) as 2*sigmoid(2g)-1 inside one wide Sigmoid instruction
    wg[:, 2 * H : 3 * H] *= 2.0
    wg = wg.astype(NPBF16)

    wfc = np.concatenate([W_fc.T, b_fc[None, :]], axis=0).astype(NPBF16)  # (65, 8)
    yb = np.zeros((128, 1), dtype=np.float32)
    yb[0:C, 0] = b_fc
    yb[32 : 32 + C, 0] = b_fc

    bpc = x.shape[0] // ncores
    in_maps = []
    for k in range(ncores):
        xc = x[k * bpc : (k + 1) * bpc]              # (bpc, T, N, C)
        xt = xc.transpose(3, 1, 0, 2).reshape(C, T, seqs)
        xext = np.empty((KX, T, seqs), dtype=NPBF16)
        xext[0:C] = xt.astype(NPBF16)
        xext[C] = np.ones((T, seqs), dtype=NPBF16)
        in_maps.append({"xin": xext, "wg": wg, "wfc": wfc, "yb": yb})
    return in_maps


_CACHE = {}


def _get_nc():
    if "nc" not in _CACHE:
        _CACHE["nc"] = build_nc()
    return _CACHE["nc"]


def kernel(x, W_ih, W_hh, b_ih, b_hh, W_fc, b_fc, **run_kwargs):
    nc = _get_nc()
    in_maps = prep_inputs(x, W_ih, W_hh, b_ih, b_hh, W_fc, b_fc)
    res = run_bass_kernel_spmd(nc, in_maps, list(range(NCORES)), **run_kwargs)
    outs = res.results
    ys = []
    for k in range(NCORES):
        yk = np.asarray(outs[k]["y"])               # (C, SEQS) f32
        ys.append(yk.T.reshape(BPC, N, C))
    y = np.concatenate(ys, axis=0)                  # (B, N, C)
    if run_kwargs.get("trace"):
        _CACHE["last_result"] = res
    return y.astype(np.float32)



# revision 29
# speedup vs baseline: 1.0128x; 1.0038x over previous
"""Trainium2 Bass kernel for nn_BasicLSTM: (B,T,N,C) shared-weight LSTM -> FC.

Strategy (data parallel over 8 cores, B=64 -> 8 batches/core). The kernel is
ScalarE(ACT)-throughput-bound (~0.81ns per free-dim column, all activations),
so the design minimizes ACT columns and ACT instruction count:
  - seqs = 8*1370 = 10960 independent (b,n) sequences per core, T=12, C=8,
    H=64. Layout "gates on partitions, sequences on free dim"; 11 pairs of
    two 512-seq blocks (block0 -> partitions 0:64, block1 -> 64:128).
  - Per (pair, step): 8 matmuls (4 gates x 2 blocks, lhsT = [W_hh.T; W_ih.T;
    b], 73x64) fill ONE 4-bank PSUM tile [i|f|o|2g]; a single wide Sigmoid
    (FD 2048) produces all four gates: tanh(g) = 2*sigmoid(2g) - 1 with the
    g weights host-prescaled by 2 (the affine is a 4x-mode DVE tensor_scalar).
  - Cell state for ALL pairs lives in one big SBUF buffer; tanh(c) runs as 2
    wide chunk instructions per step (vs 11 narrow ones), software-pipelined
    so each chunk's producers finished a couple of sigmoids earlier.
  - h = sigmoid(o)*tanh(c) is written by DVE directly into the next step's
    rhs tile [h(0:64); x_t(64:72); ones(72)]; block1's h moves partitions
    via a SBUF->SBUF DMA (DVE lanes cannot cross partition offsets; matmul
    lhsT/rhs base partitions must be 0 for K>64, so no flipped layout).
  - x arrives host-pretransposed as (9, T, seqs) bf16 (channel 8 = ones row
    carries the gate biases through the contraction); one x-DMA per pair.
  - Last step: per-pair tanh(c) + FC (y = W_fc @ h_T, K=64, bias via a
    per-partition tensor_scalar) pipelined 4+ pairs behind so the h-store
    DMA chain never stalls the tensor queue; block1's FC reads h directly
    from its SBUF tile with a base-64 lhsT copy of W_fc.
  - Ramp: small-weight + first-x DMAs lead the sync queue; 3 warm-up
    matmuls spin the PE toward max pstate under the DMA shadow.
Measured: 374us (baseline) -> ~347us; rel err 6.6e-3 (fp32 ref, bf16 data).
"""

import os
from contextlib import ExitStack

import numpy as np

import concourse.bass as bass
import concourse.mybir as mybir
import concourse.tile as tile
from concourse import bacc
from concourse.bass_utils import run_bass_kernel_spmd
from concourse.tile import add_dep_helper

B, T, N, C, H = 64, 12, 1370, 8, 64
NCORES = 8
BPC = B // NCORES          # batches per core
SEQS = BPC * N             # 10960 sequences per core
S = 512                    # sequence block size (free dim per matmul)
KH = H                     # 64 rows of h in rhs
KX = C + 1                 # 8 x-channels + ones row
K = KH + KX                # 73
G4 = 4 * H                 # 256

BF16 = mybir.dt.bfloat16
F32 = mybir.dt.float32
CDT = BF16  # cell-state dtype (bf16 doubles DVE throughput on the c update)
NPBF16 = mybir.dt.np(BF16)

AF = mybir.ActivationFunctionType


def _blocks(seqs: int, s: int):
    """Blocks of width <= s covering seqs with the remainder split evenly
    over the last two blocks (keeps every ACT/DVE op at its true width
    instead of streaming padding)."""
    nfull = seqs // s
    rem = seqs - nfull * s
    if rem == 0:
        widths = [s] * nfull
    else:
        # steal one full block so the tail splits into two equal-ish blocks
        tail = s + rem
        widths = [s] * (nfull - 1) + [(tail + 1) // 2, tail // 2]
    out = []
    c0 = 0
    for w in widths:
        out.append((c0, w))
        c0 += w
    return out


def _fence(nc, producers):
    """The TRN2 Matmult ISA slot holds a single sync-wait, but the first
    matmul of a step naturally depends on 3-4 engines (ACT psum release, DVE
    h-write, DMA x / h-move). Funnel those deps through a chain of SyncE nops
    (one wait each); callers then depend only on the returned nop."""
    last = None
    seen = set()
    for p in producers:
        if p is None:
            continue
        pi = p.ins if hasattr(p, "ins") else p
        if id(pi) in seen:
            continue
        seen.add(id(pi))
        n = nc.sync.nop(nofuse=True, hint="depfence")
        add_dep_helper(n.ins, pi, reason="dep fence")
        last = n
    return last


def build_nc(seqs: int = SEQS, s: int = S, use_fence: bool = False) -> bass.Bass:
    nc = bacc.Bacc("TRN2", target_bir_lowering=False, debug=False)

    xin = nc.declare_dram_parameter("xin", [KX, T, seqs], BF16, isOutput=False)
    wg = nc.declare_dram_parameter("wg", [K, G4], BF16, isOutput=False)
    wfc = nc.declare_dram_parameter("wfc", [H + 1, C], BF16, isOutput=False)
    yb = nc.declare_dram_parameter("yb", [128, 1], F32, isOutput=False)
    y = nc.declare_dram_parameter("y", [C, seqs], F32, isOutput=True)

    blocks = _blocks(seqs, s)
    pairs = [blocks[i : i + 2] for i in range(0, len(blocks), 2)]

    with tile.TileContext(nc) as tc, ExitStack() as ctx:
        const = ctx.enter_context(tc.tile_pool(name="const", bufs=1))
        rhsp = ctx.enter_context(tc.tile_pool(name="rhs", bufs=36))
        sgp = ctx.enter_context(tc.tile_pool(name="sg", bufs=12))
        vgp = ctx.enter_context(tc.tile_pool(name="vg", bufs=6))
        igp = ctx.enter_context(tc.tile_pool(name="ig", bufs=6))
        fcpool = ctx.enter_context(tc.tile_pool(name="fcp", bufs=6))
        thcp = ctx.enter_context(tc.tile_pool(name="thcl", bufs=4))
        htp = ctx.enter_context(tc.tile_pool(name="htmp", bufs=7))
        ysp = ctx.enter_context(tc.tile_pool(name="ys", bufs=2))
        # one 4-bank PSUM tile [i|f|o|2g] per pair, double-buffered = all 8
        # banks; the FC tail's pf tiles borrow slots from the same pool
        pifo = ctx.enter_context(tc.tile_pool(name="pifo", bufs=2, space="PSUM"))

        w_sb = const.tile([K, G4], BF16)
        # x-projection rows first: the t=0 matmuls need only rows KH:K, so
        # they can launch before the bulk of the weights arrive; the first
        # pair's t=0 x ride right behind it on the sync queue
        nc.sync.dma_start(out=w_sb[KH:K, :], in_=wg[KH:K, :])

        wfc_sb = const.tile([H + 1, C], BF16)
        # W_fc.T replicated at partitions 64:128: lets the T-1 FC matmul for
        # block1 read h straight out of the ht tile (partitions 64:128),
        # skipping the h-store DMA hop on the critical tail
        wfc64_sb = const.tile([128, C], BF16)
        yb_sb = const.tile([128, 1], F32)
        # cell state + tanh(c) for ALL pairs live in two big SBUF buffers so
        # the per-step tanh(c) runs as 2 wide ACT instructions instead of 11
        # narrow ones (saves ~180ns/inst of ACT access overhead)
        cbuf = const.tile([128, seqs // 2], CDT)
        thcbuf = const.tile([128, seqs // 2], BF16)
        # compact: only block0's h goes through hstore (block1 h is read by
        # the FC straight from its ht tile), indexed by the pair offset
        hstore = const.tile([H, seqs // 2], BF16)
        scratch = const.tile([128, s], BF16)

        # gate column offsets in w_sb: pytorch order i, f, g, o
        WI, WF, WG, WO = 0, H, 2 * H, 3 * H

        MULT = mybir.AluOpType.mult
        ADDOP = mybir.AluOpType.add

        def phase1(st, t):
            """Gate matmuls, one sigmoid over [i|f|o|2g], cell update -> cbuf.

            tanh(g) = 2*sigmoid(2g) - 1; the host pre-scales the g-gate
            weights by 2, so a single wide Sigmoid covers all four gates and
            a cheap 4x-mode tensor_scalar recovers tanh(g) on the DVE."""
            pair, rhs_t, off = st["pair"], st["rhs"], st["off"]
            wd = pair[0][1]  # both blocks in a pair have equal width
            ifo = pifo.tile([128, 4 * s], F32, name="ifo", tag="ifo")

            # matmuls; gate sections bank-aligned at stride s (a PSUM matmul
            # dst must not cross a 2KB bank boundary -- verified wrong on HW
            # otherwise); the g section sits last so the sigmoid can stop at
            # 3*s+wd and skip the tail pair's trailing padding
            for gof, dsec in ((WG, 3), (WI, 0), (WF, 1), (WO, 2)):
                for blk, (c0, bwd) in enumerate(pair):
                    pb = 64 * blk
                    b0 = blk * pair[0][1]
                    if t == 0:
                        lh = w_sb[KH:K, gof : gof + H]
                        rh = rhs_t[t][KH:K, b0 : b0 + bwd]
                    else:
                        lh = w_sb[:, gof : gof + H]
                        rh = rhs_t[t][:, b0 : b0 + bwd]
                    nc.tensor.matmul(
                        ifo[pb : pb + 64, dsec * s : dsec * s + bwd], lh, rh
                    )

            sg = sgp.tile([128, 4 * s], BF16, name="sg", tag="sg")
            nc.scalar.activation(
                sg[:, 0 : 3 * s + wd], ifo[:, 0 : 3 * s + wd], AF.Sigmoid
            )
            st["sg"] = sg
            # tanh(g) = 2*sigmoid(2g) - 1 on the DVE (4x mode)
            vg = vgp.tile([128, s], BF16, name="vg", tag="vg")
            nc.vector.tensor_scalar(
                vg[:, 0:wd], sg[:, 3 * s : 3 * s + wd], 2.0, -1.0, MULT, ADDOP
            )

            c_t = cbuf[:, off : off + wd]
            if t == 0:
                nc.vector.tensor_mul(c_t, sg[:, 0:wd], vg[:, 0:wd])
            else:
                ig = igp.tile([128, s], BF16, name="ig", tag="ig")
                nc.vector.tensor_mul(ig[:, 0:wd], sg[:, 0:wd], vg[:, 0:wd])
                fc = fcpool.tile([128, s], CDT, name="fc", tag="fc")
                nc.vector.tensor_mul(fc[:, 0:wd], sg[:, s : s + wd], c_t)
                nc.vector.tensor_add(c_t, ig[:, 0:wd], fc[:, 0:wd])

        def phase2(st, t, thc=None, thc_off=None):
            """h = sigmoid(o) * tanh(c) into next-step rhs (or hstore)."""
            pair, rhs_t, off, sg = st["pair"], st["rhs"], st["off"], st["sg"]
            if thc is None:
                thc, thc_off = thcbuf, off
            for blk, (c0, wd) in enumerate(pair):
                pb = 64 * blk
                so = sg[pb : pb + 64, 2 * s : 2 * s + wd]
                tc_half = thc[pb : pb + 64, thc_off : thc_off + wd]
                b0 = blk * pair[0][1]
                if blk == 0:
                    dst = (hstore[0:H, off : off + wd] if t == T - 1
                           else rhs_t[t + 1][0:KH, b0 : b0 + wd])
                    nc.vector.tensor_mul(dst, so, tc_half)
                else:
                    ht = htp.tile([128, s], BF16, name="ht", tag="ht")
                    nc.vector.tensor_mul(ht[pb : pb + 64, 0:wd], so, tc_half)
                    if t == T - 1:
                        st["ht"] = ht  # FC reads h directly from here
                    else:
                        nc.sync.dma_start(
                            out=rhs_t[t + 1][0:KH, b0 : b0 + wd],
                            in_=ht[pb : pb + 64, 0:wd],
                        )

        def emit_fc(sts):
            """FC for up to two pairs: one pf tile + one wide bias-add, so
            the PSUM-pool rotation only pays one extra slot per duo. Block0
            reads hstore; block1 reads its ht tile directly (base-64 lhsT)."""
            pf = pifo.tile([128, 4 * s], F32, tag="ifo", name="pf")
            blks = []
            for d, st in enumerate(sts):
                for blk, (c0, wd) in enumerate(st["pair"]):
                    blks.append((c0, wd, blk, d, st))
            for c0, wd, blk, d, st in blks:
                if blk == 0:
                    nc.tensor.matmul(
                        pf[0:C, d * s : d * s + wd],
                        wfc_sb[0:H, :],
                        hstore[:, st["off"] : st["off"] + wd],
                    )
                else:
                    nc.tensor.matmul(
                        pf[32 : 32 + C, d * s : d * s + wd],
                        wfc64_sb[64:128, :],
                        st["ht"][64:128, 0:wd],
                    )
            yt = ysp.tile([128, 4 * s], F32, name="yt", tag="yt")
            nw = len(sts) * s
            nc.vector.tensor_scalar(
                yt[0:40, 0:nw], pf[0:40, 0:nw], yb_sb[0:40, 0:1], None, ADDOP
            )
            if len(sts) == 1:
                # final pair: ACT's sequencer is idle by now -- issue both
                # y-DMAs in parallel on otherwise-free queues
                engs = [nc.scalar, nc.sync]
            else:
                engs = [nc.sync, nc.gpsimd]
            for j, (c0, wd, blk, d, st) in enumerate(blks):
                engs[j % 2].dma_start(
                    out=y[:, c0 : c0 + wd],
                    in_=yt[32 * blk : 32 * blk + C, d * s : d * s + wd],
                )

        # process pairs in interleaved groups so several independent
        # recurrences keep every engine busy (and TensorE HAM-warm)
        GROUP = 11
        PREFETCH = 2

        def alloc_rhs(st, t, eng=None):
            pair, rhs_t = st["pair"], st["rhs"]
            pw = sum(w for _, w in pair)
            rt = rhsp.tile([K, 2 * s], BF16, name="rhs", tag="rhs")
            rhs_t[t] = rt
            # one DMA covers both blocks' x (halves the DMA-issue load)
            (eng or nc.gpsimd).dma_start(
                out=rt[KH:K, 0:pw],
                in_=xin[:, t, pair[0][0] : pair[0][0] + pw],
            )

        for g0 in range(0, len(pairs), GROUP):
            grp = pairs[g0 : g0 + GROUP]
            states = []
            off = 0
            for pair in grp:
                states.append({"pair": pair, "rhs": [None] * T, "off": off})
                off += pair[0][1]
            # first pair's t=0 x as sync DMA #2 (right behind the small wg)
            alloc_rhs(states[0], 0, nc.sync)
            # remaining const DMAs now that the critical two are in flight
            nc.sync.dma_start(out=w_sb[0:KH, :], in_=wg[0:KH, :])
            nc.sync.dma_start(out=wfc_sb[:, :], in_=wfc[:, :])
            nc.sync.dma_start(out=wfc64_sb[64:128, :], in_=wfc[0:H, :])
            nc.sync.dma_start(out=yb_sb[:, :], in_=yb[:, :])
            # PE warm-up (gpsimd memset frees the DVE and starts ~1us sooner)
            nc.gpsimd.memset(scratch[:, :], 1.0)
            nc.scalar.activation(scratch[0:1, 0:8], scratch[0:1, 0:8], AF.Sigmoid)
            wmup = pifo.tile([128, 4 * s], F32, name="wmup", tag="ifo")
            for wi in range(3):
                nc.tensor.matmul(
                    wmup[0:64, 0:s], scratch[0:73, 0:64], scratch[0:73, 0:s]
                )
            for pi, st in enumerate(states):
                for t in range(PREFETCH):
                    if (pi, t) != (0, 0):
                        alloc_rhs(st, t)
            # Software-pipelined schedule. The batched tanh(c) is split in two
            # chunks, and each chunk's ACT instruction is emitted one-or-two
            # sigmoids AFTER its last producer pair so the DVE cell-update
            # latency hides behind ACT work (no ACT gap waiting on the adds).
            # The first two pairs of step t+1 are emitted before chunk B of
            # step t for the same reason.
            mid = (len(states) + 1) // 2
            A, Bc = states[:mid], states[mid:]

            def emit_phase1(st, t):
                if t + PREFETCH < T:
                    alloc_rhs(st, t + PREFETCH)
                phase1(st, t)

            def chunk_tanh(grp, t):
                lo = grp[0]["off"]
                hi = grp[-1]["off"] + grp[-1]["pair"][0][1]
                nc.scalar.activation(thcbuf[:, lo:hi], cbuf[:, lo:hi], AF.Tanh)

            for t in range(T - 1):
                for st in states[0 if t == 0 else 2 : mid]:
                    emit_phase1(st, t)
                emit_phase1(states[mid], t)  # p_mid before chunk A's tanh
                chunk_tanh(A, t)
                for st in A:
                    phase2(st, t)
                for st in states[mid + 1 :]:
                    emit_phase1(st, t)
                for st in states[0:2]:  # head of step t+1 before chunk B
                    emit_phase1(st, t + 1)
                chunk_tanh(Bc, t)
                for st in Bc:
                    phase2(st, t)
            # last step: per-pair tanh(c), pipelined with lags -- tanh(p) rides
            # one sigmoid behind phase1(p+1) (so the DVE cell update is done),
            # and FC(p) rides FC_LAG pairs behind (so its h-store DMA chain,
            # incl the ~900ns sem propagation, is complete -> no tensor stall)
            FC_LAG = 4

            def tanh_phase2(st):
                wd = st["pair"][0][1]
                thc = thcp.tile([128, s], BF16, name="thc", tag="thc")
                nc.scalar.activation(
                    thc[:, 0:wd],
                    cbuf[:, st["off"] : st["off"] + wd],
                    AF.Tanh,
                )
                phase2(st, T - 1, thc=thc, thc_off=0)

            # p0/p1 phase1 ran during step T-2's tail; the per-pair tanh
            # rides TWO sigmoids behind phase1 so its cell-update never waits
            # the chunk-B(T-2) read of cbuf (tile-granular WAR)
            for i in range(2, len(states)):
                phase1(states[i], T - 1)
                tanh_phase2(states[i - 2])
                if i == 7:
                    emit_fc(states[0:4])
                elif i == 9:
                    emit_fc(states[4:8])
            tanh_phase2(states[-2])
            emit_fc(states[8:10])
            tanh_phase2(states[-1])
            emit_fc(states[10:11])


    nc.compile()
    return nc


def prep_inputs(x, W_ih, W_hh, b_ih, b_hh, W_fc, b_fc, seqs=SEQS, ncores=NCORES):
    """Host-side shard + transpose + weight packing. Returns in_maps."""
    x = np.asarray(x, dtype=np.float32)
    W_ih = np.asarray(W_ih, dtype=np.float32)
    W_hh = np.asarray(W_hh, dtype=np.float32)
    b = np.asarray(b_ih, dtype=np.float32) + np.asarray(b_hh, dtype=np.float32)
    W_fc = np.asarray(W_fc, dtype=np.float32)
    b_fc = np.asarray(b_fc, dtype=np.float32)

    wg = np.zeros((K, G4), dtype=np.float32)
    for g in range(4):
        rows = slice(H * g, H * g + H)
        wg[0:KH, H * g : H * g + H] = W_hh[rows, :].T
        wg[KH : KH + C, H * g : H * g + H] = W_ih[rows, :].T
        wg[K - 1, H * g : H * g + H] = b[rows]
    # g-gate (pytorch gate index 2) pre-scaled by 2: the kernel evaluates
    # tanh(g) as 2*sigmoid(2g)-1 inside one wide Sigmoid instruction
    wg[:, 2 * H : 3 * H] *= 2.0
    wg = wg.astype(NPBF16)

    wfc = np.concatenate([W_fc.T, b_fc[None, :]], axis=0).astype(NPBF16)  # (65, 8)
    yb = np.zeros((128, 1), dtype=np.float32)
    yb[0:C, 0] = b_fc
    yb[32 : 32 + C, 0] = b_fc

    bpc = x.shape[0] // ncores
    in_maps = []
    for k in range(ncores):
        xc = x[k * bpc : (k + 1) * bpc]              # (bpc, T, N, C)
        xt = xc.transpose(3, 1, 0, 2).reshape(C, T, seqs)
        xext = np.empty((KX, T, seqs), dtype=NPBF16)
        xext[0:C] = xt.astype(NPBF16)
        xext[C] = np.ones((T, seqs), dtype=NPBF16)
        in_maps.append({"xin": xext, "wg": wg, "wfc": wfc, "yb": yb})
    return in_maps


_CACHE = {}


def _get_nc():
    if "nc" not in _CACHE:
        _CACHE["nc"] = build_nc()
    return _CACHE["nc"]


def kernel(x, W_ih, W_hh, b_ih, b_hh, W_fc, b_fc, **run_kwargs):
    nc = _get_nc()
    in_maps = prep_inputs(x, W_ih, W_hh, b_ih, b_hh, W_fc, b_fc)
    res = run_bass_kernel_spmd(nc, in_maps, list(range(NCORES)), **run_kwargs)
    outs = res.results
    ys = []
    for k in range(NCORES):
        yk = np.asarray(outs[k]["y"])               # (C, SEQS) f32
        ys.append(yk.T.reshape(BPC, N, C))
    y = np.concatenate(ys, axis=0)                  # (B, N, C)
    if run_kwargs.get("trace"):
        _CACHE["last_result"] = res
    return y.astype(np.float32)

